# revision 22
# baseline (speedup 1.0000x reference)
"""Distributed causal MHA + RoPE kernel for one TRN2 chip (8 NeuronCores).

Problem: x[4, 2048, 1024] f32, wqkv[3072, 1024], wout[1024, 1024],
16 heads, d_head 64, causal, RoPE theta 1e4.

Sharding: core = 2*b + g  (b = batch 0..3, g = head-group 0..1).
Each core computes heads 8g..8g+7 of batch b for the full sequence:
  - QK^T projection in transposed layout (head-dims on partitions,
    positions on free axis), per-head dims permuted [evens|odds] so RoPE
    is plain elementwise ops on contiguous partition ranges.
  - V projection in natural layout [s, dims] with an appended ones
    column, so the PV matmul also produces softmax denominators.
  - S^T = K^T.T @ Q^T per [128k x 512q] chunk, causal chunks skipped,
    exp on ScalarE with the 1/8 scale folded in (no max subtraction --
    scores are provably < ~3 for this data distribution), diagonal
    triangle zeroed with one affine_select per diagonal chunk.
  - O^T accumulated in PSUM, normalized by broadcasted reciprocal
    denominators, written as bf16 y^T.
  - Pairwise (within-batch) AllGather of y^T, overlapped with the next
    phase's attention; out[:, 512g:512g+512] = y_full @ wout.T[:, slice].

v2 schedule: V-projection first (smallest DMA gate), phase order
3,0,2,1 (each AllGather hides under the next phase), one fused
software-pipelined chunk stream per phase with qk-projections /
out-projections injected at head-pair boundaries, and the final phase's
AllGather split in two halves so the tail only waits on the last
128-row exchange.
Output per core: [2048, 512] f32; host concatenates.
"""

import numpy as np
import ml_dtypes

try:
    import concourse.bass as bass
except ImportError:  # staged repo path inside the container
    import sys

    sys.path.insert(0, "/opt/trn_rl_repo")
    import concourse.bass as bass

import concourse.bacc as bacc
import concourse.mybir as mybir
import concourse.tile as tile
from concourse.bass_utils import run_bass_kernel_spmd

bf16 = ml_dtypes.bfloat16

B, S, D, H, DH = 4, 2048, 1024, 16, 64
HPC = H // 2          # heads per core = 8
QD = HPC * DH         # per-core q (or k, or v) dims = 512
NCORES = 8
ROPE_THETA = 10000.0
QCH = 512             # q chunk (matmul moving free dim)
KCH = 128             # k chunk (psum partition dim)
NQC = S // QCH        # 4
NKC = S // KCH        # 16
DCH = D // 128        # 8 contraction chunks for the projections

F32 = mybir.dt.float32
BF16 = mybir.dt.bfloat16

RG = [[0, 1], [2, 3], [4, 5], [6, 7]]


def _build(nc, tc, ctx):
    # ---------------- I/O ----------------
    xT = nc.declare_dram_parameter("xT", [D, S], BF16, isOutput=False)
    wqkT = nc.declare_dram_parameter("wqkT", [D, 2 * QD], BF16, isOutput=False)
    wvT = nc.declare_dram_parameter("wvT", [D, QD], BF16, isOutput=False)
    woutT = nc.declare_dram_parameter("woutT", [D, QD], BF16, isOutput=False)
    cosT = nc.declare_dram_parameter("cosT", [128, S], BF16, isOutput=False)
    sinT = nc.declare_dram_parameter("sinT", [128, S], BF16, isOutput=False)
    out_ext = nc.declare_dram_parameter("out", [S, QD], F32, isOutput=True)

    # AllGather bounce buffers. Full-phase AGs for qc 3,0,2; the final
    # phase (qc=1) exchanges in two halves (hp01 then hp23).
    ag_in = []
    ag_out = []
    for ph in range(NQC):
        ag_in.append(
            tc.tile([QD, QCH], BF16, space="DRAM", name=f"ag_in{ph}", bufs=1)[0]
        )
        ag_out.append(
            tc.tile(
                [2 * QD, QCH],
                BF16,
                space="DRAM",
                addr_space="Shared",
                name=f"ag_out{ph}",
                bufs=1,
            )[0]
        )
    ag_out_fa = tc.tile(
        [6 * KCH, QCH], BF16, space="DRAM", addr_space="Shared", name="ag_out_fa", bufs=1
    )[0]
    ag_out_fb = tc.tile(
        [2 * KCH, QCH], BF16, space="DRAM", addr_space="Shared", name="ag_out_fb", bufs=1
    )[0]
    # Tiny warm-up collective: the first collective of the NEFF absorbs the
    # cross-core launch skew + ncfw setup (~40-60us observed). Fire it at
    # t~0 with no data dependencies so the cost overlaps the projections.
    ag_warm_in = tc.tile([1, 16], BF16, space="DRAM", name="ag_warm_in", bufs=1)[0]
    ag_warm_out = tc.tile(
        [2, 16], BF16, space="DRAM", addr_space="Shared", name="ag_warm_out", bufs=1
    )[0]

    # ---------------- SBUF pools ----------------
    const_pool = ctx.enter_context(tc.tile_pool(name="consts", bufs=1))
    xt_sb = [const_pool.tile([128, S], BF16, name=f"xt{i}") for i in range(DCH)]
    wqk_sb = [const_pool.tile([128, 2 * QD], BF16, name=f"wqk{i}") for i in range(DCH)]
    wv_sb = [const_pool.tile([128, QD], BF16, name=f"wv{i}") for i in range(DCH)]
    wout_sb = [const_pool.tile([128, QD], BF16, name=f"wo{i}") for i in range(DCH)]
    cos_sb = const_pool.tile([128, S], BF16, name="cos")
    sin_sb = const_pool.tile([128, S], BF16, name="sin")

    # --- initial DMA, priming order: v-proj inputs first, wout last.
    # Issues are spread across the sync/scalar/vector descriptor queues
    # (each dma_start costs ~0.7us of issue time on its queue), and the
    # first tiles are split by partition so the very first v-proj matmul
    # isn't gated on a single 128-descriptor queue drain.
    for i in range(DCH):
        r = slice(128 * i, 128 * (i + 1))
        if i < 2:  # first-bite: halve so two queues carry each tile
            lo, hi = 128 * i, 128 * i + 64
            nc.sync.dma_start(out=wv_sb[i][0:64, :], in_=wvT[lo:hi, :])
            nc.sync.dma_start(out=wv_sb[i][64:128, :], in_=wvT[hi : hi + 64, :])
            nc.sync.dma_start(out=xt_sb[i][0:64, 0:QCH], in_=xT[lo:hi, 0:QCH])
            nc.sync.dma_start(
                out=xt_sb[i][64:128, 0:QCH], in_=xT[hi : hi + 64, 0:QCH]
            )
        else:
            nc.sync.dma_start(out=wv_sb[i], in_=wvT[r, :])
            nc.sync.dma_start(out=xt_sb[i][:, 0:QCH], in_=xT[r, 0:QCH])
    for c in (1, 2):  # gpsimd queue: xt columns for v-proj sc 4..11
        for i in range(DCH):
            nc.gpsimd.dma_start(
                out=xt_sb[i][:, QCH * c : QCH * (c + 1)],
                in_=xT[128 * i : 128 * (i + 1), QCH * c : QCH * (c + 1)],
            )
    for c in (0, 2, 1, 3):  # scalar queue: wqk (pairs 0,1 first)
        for i in range(DCH):
            nc.scalar.dma_start(
                out=wqk_sb[i][:, 256 * c : 256 * (c + 1)],
                in_=wqkT[128 * i : 128 * (i + 1), 256 * c : 256 * (c + 1)],
            )
    for i in range(DCH):  # sync queue: xt c3, then rope tables, wout last
        nc.sync.dma_start(
            out=xt_sb[i][:, 3 * QCH : 4 * QCH],
            in_=xT[128 * i : 128 * (i + 1), 3 * QCH : 4 * QCH],
        )
    for c in range(4):
        nc.sync.dma_start(
            out=cos_sb[:, QCH * c : QCH * (c + 1)],
            in_=cosT[:, QCH * c : QCH * (c + 1)],
        )
        nc.sync.dma_start(
            out=sin_sb[:, QCH * c : QCH * (c + 1)],
            in_=sinT[:, QCH * c : QCH * (c + 1)],
        )
    for i in range(DCH):
        nc.sync.dma_start(out=wout_sb[i], in_=woutT[128 * i : 128 * (i + 1), :])

    # lower-triangular (keep q >= k) [128, 128] mask, built once
    tri_sb = const_pool.tile([KCH, KCH], BF16, name="tri")
    nc.vector.memset(tri_sb, 1.0)
    nc.gpsimd.affine_select(
        out=tri_sb,
        in_=tri_sb,
        pattern=[[1, KCH]],
        channel_multiplier=-1,
        base=0,
        compare_op=mybir.AluOpType.is_ge,
        fill=0.0,
    )

    # persistent activation buffers
    qkrot_pool = ctx.enter_context(tc.tile_pool(name="qkrot", bufs=1))
    qkrot = [qkrot_pool.tile([128, S], BF16, name=f"qkrot{m}") for m in range(8)]
    vaug_pool = ctx.enter_context(tc.tile_pool(name="vaug", bufs=1))
    v_sb = [vaug_pool.tile([128, HPC, DH + 1], BF16, name=f"v{sc}") for sc in range(16)]
    ybuf_pool = ctx.enter_context(tc.tile_pool(name="ybuf", bufs=1))
    ybuf = [ybuf_pool.tile([128, S], BF16, name=f"y{hp}") for hp in range(4)]

    # transient pools
    spool = ctx.enter_context(tc.tile_pool(name="spsum", bufs=2, space="PSUM"))
    opool = ctx.enter_context(tc.tile_pool(name="opsum", bufs=4, space="PSUM"))
    qkraw_pool = ctx.enter_context(tc.tile_pool(name="qkraw", bufs=2))
    rope_tmp = ctx.enter_context(tc.tile_pool(name="ropetmp", bufs=2))
    p_pool = ctx.enter_context(tc.tile_pool(name="ptiles", bufs=6))
    r_pool = ctx.enter_context(tc.tile_pool(name="rbcast", bufs=2))
    yf_pool = ctx.enter_context(tc.tile_pool(name="yfull", bufs=2))
    osb_pool = ctx.enter_context(tc.tile_pool(name="outsb", bufs=3))
    warm_pool = ctx.enter_context(tc.tile_pool(name="warm", bufs=1))

    # preload the scalar-engine exp table before any real work
    def prewarm_exp():
        wsrc = warm_pool.tile([1, 8], F32, name="wsrc")
        wdst = warm_pool.tile([1, 8], BF16, name="wdst")
        nc.vector.memset(wsrc, 0.0)
        nc.scalar.activation(
            out=wdst, in_=wsrc,
            func=mybir.ActivationFunctionType.Exp, scale=0.125,
        )

    # ---------------- V projection (natural layout + ones column) -------
    def v_proj():
        for sc in range(16):
            ps = opool.tile([128, QD], F32, tag="ot", name=f"psv_{sc}")
            for kc in range(DCH):
                nc.tensor.matmul(
                    ps,
                    lhsT=xt_sb[kc][:, 128 * sc : 128 * (sc + 1)],
                    rhs=wv_sb[kc],
                    start=(kc == 0),
                    stop=(kc == DCH - 1),
                )
            nc.vector.tensor_copy(
                out=v_sb[sc][:, :, 0:DH],
                in_=ps.rearrange("p (h d) -> p h d", h=HPC),
            )
            nc.vector.memset(v_sb[sc][:, :, DH : DH + 1], 1.0)

    # ---------------- QK^T projection + RoPE for head-pair j ----------------
    def qk_proj_pair(j):
        raws = {}
        for m in (j, 4 + j):
            raws[m] = qkraw_pool.tile([128, S], BF16, tag="raw", name=f"raw{m}")
        for m in (j, 4 + j):
            # nch pairs share the stationary wqk slice per kc (the two
            # back-to-back matmuls reuse the loaded weights)
            for n0 in (0, 2):
                ps = [
                    opool.tile([128, QCH], F32, tag="ot", name=f"psqk_{m}_{n0 + d}")
                    for d in range(2)
                ]
                for kc in range(DCH):
                    for d in range(2):
                        nc.tensor.matmul(
                            ps[d],
                            lhsT=wqk_sb[kc][:, 128 * m : 128 * (m + 1)],
                            rhs=xt_sb[kc][
                                :, QCH * (n0 + d) : QCH * (n0 + d + 1)
                            ],
                            start=(kc == 0),
                            stop=(kc == DCH - 1),
                        )
                for d in range(2):
                    nc.scalar.copy(
                        out=raws[m][:, QCH * (n0 + d) : QCH * (n0 + d + 1)],
                        in_=ps[d],
                    )
        for m in (j, 4 + j):
            raw = raws[m]
            # RoPE, 6 DVE ops: rot = raw*cosF + pS (sin signs folded in sin_sb)
            pC = rope_tmp.tile([128, S], BF16, tag="pC", name=f"pC_{m}")
            pS = rope_tmp.tile([128, S], BF16, tag="pS", name=f"pS_{m}")
            nc.vector.tensor_mul(pC, raw, cos_sb)
            nc.vector.tensor_mul(pS[0:32, :], raw[32:64, :], sin_sb[32:64, :])
            nc.vector.tensor_mul(pS[32:64, :], raw[0:32, :], sin_sb[0:32, :])
            nc.vector.tensor_mul(pS[64:96, :], raw[96:128, :], sin_sb[96:128, :])
            nc.vector.tensor_mul(pS[96:128, :], raw[64:96, :], sin_sb[64:96, :])
            nc.vector.tensor_add(qkrot[m], pC, pS)

    # ---------------- attention phase: fused pipelined chunk stream ------
    # One phase = all 4 head-pairs of q-chunk qc, software-pipelined with
    # lag 1 ACROSS hp boundaries:  S(i) exp(i) PV(i-1) ... so the scalar
    # engine always has exp work one chunk ahead of the PE's PV.
    # `inject[hp]` closures (projection / out-projection blocks) are
    # emitted right after hp's normalize, at which point that hp's PSUM
    # accumulators have been freed.
    def attn_phase(qc, inject=None):
        inject = inject or {}
        nkc = 4 * qc + 4
        chunks = [(hp, kc) for hp in range(4) for kc in range(nkc)]
        o_ps = {}
        s_tiles = {}
        p_tiles = {}

        def emit_S(hp, kc):
            c0 = max(0, KCH * kc - QCH * qc)
            s_ps = spool.tile(
                [128, 2 * QCH], F32, tag="stile", name=f"s_{qc}_{hp}_{kc}"
            )
            for ab in range(2):
                lo = 64 * ab
                nc.tensor.matmul(
                    s_ps[:, QCH * ab + c0 : QCH * (ab + 1)],
                    lhsT=qkrot[4 + hp][lo : lo + 64, KCH * kc : KCH * (kc + 1)],
                    rhs=qkrot[hp][lo : lo + 64, QCH * qc + c0 : QCH * (qc + 1)],
                    start=True,
                    stop=True,
                )
            s_tiles[(hp, kc)] = s_ps

        def emit_exp(hp, kc):
            c0 = max(0, KCH * kc - QCH * qc)
            s_ps = s_tiles.pop((hp, kc))
            p_t = p_pool.tile(
                [128, 2 * QCH], BF16, tag="ptile", name=f"p_{qc}_{hp}_{kc}"
            )
            if c0 == 0:
                nc.scalar.activation(
                    out=p_t, in_=s_ps,
                    func=mybir.ActivationFunctionType.Exp, scale=0.125,
                )
            else:
                for ab in range(2):
                    nc.scalar.activation(
                        out=p_t[:, QCH * ab + c0 : QCH * (ab + 1)],
                        in_=s_ps[:, QCH * ab + c0 : QCH * (ab + 1)],
                        func=mybir.ActivationFunctionType.Exp, scale=0.125,
                    )
            if KCH * kc >= QCH * qc:  # diagonal: zero k > q
                for ab in range(2):
                    nc.vector.tensor_mul(
                        p_t[:, QCH * ab + c0 : QCH * ab + c0 + KCH],
                        p_t[:, QCH * ab + c0 : QCH * ab + c0 + KCH],
                        tri_sb,
                    )
            p_tiles[(hp, kc)] = p_t

        def emit_PV(hp, kc):
            c0 = max(0, KCH * kc - QCH * qc)
            if kc == 0:
                o_ps[hp] = [
                    opool.tile([DH + 1, QCH], F32, tag="ot", name=f"o_{qc}_{hp}_{ab}")
                    for ab in range(2)
                ]
            p_t = p_tiles.pop((hp, kc))
            for ab in range(2):
                nc.tensor.matmul(
                    o_ps[hp][ab][:, c0:QCH],
                    lhsT=v_sb[kc][:, 2 * hp + ab, :],
                    rhs=p_t[:, QCH * ab + c0 : QCH * (ab + 1)],
                    start=(kc == 0),
                    stop=(kc == nkc - 1),
                )

        def emit_norm(hp):
            ops = o_ps.pop(hp)
            for ab in range(2):
                den0 = r_pool.tile([1, QCH], F32, tag="den0", name=f"dn{qc}_{hp}_{ab}")
                nc.vector.tensor_copy(out=den0, in_=ops[ab][DH : DH + 1, :])
                rsrc = r_pool.tile([1, QCH], F32, tag="rsrc", name=f"rs{qc}_{hp}_{ab}")
                nc.vector.reciprocal_approx_fast(out=rsrc, in_=den0)
                rbc = r_pool.tile([DH, QCH], F32, tag="rbc", name=f"rb{qc}_{hp}_{ab}")
                nc.gpsimd.partition_broadcast(rbc, rsrc)
                nc.vector.tensor_mul(
                    ybuf[hp][64 * ab : 64 * ab + 64, QCH * qc : QCH * (qc + 1)],
                    ops[ab][0:DH, :],
                    rbc,
                )
            nc.sync.dma_start(
                out=ag_in[qc][128 * hp : 128 * (hp + 1), :],
                in_=ybuf[hp][:, QCH * qc : QCH * (qc + 1)],
            )

        prev = None
        for hp, kc in chunks:
            emit_S(hp, kc)
            emit_exp(hp, kc)
            if prev is not None:
                emit_PV(*prev)
                if prev[1] == nkc - 1:
                    emit_norm(prev[0])
                    if prev[0] in inject:
                        inject[prev[0]]()
            prev = (hp, kc)
        emit_PV(*prev)
        emit_norm(prev[0])
        if prev[0] in inject:
            inject[prev[0]]()

    def allgather(qc):
        nc.gpsimd.collective_compute(
            "AllGather",
            mybir.AluOpType.bypass,
            replica_groups=RG,
            ins=[ag_in[qc][:, :]],
            outs=[ag_out[qc][:, :]],
        )

    def out_proj(qc):
        yf = [
            yf_pool.tile([128, QCH], BF16, tag=f"yf{i}", name=f"yf{qc}_{i}")
            for i in range(DCH)
        ]
        for i in range(DCH):
            nc.sync.dma_start(out=yf[i], in_=ag_out[qc][128 * i : 128 * (i + 1), :])
        for scl in range(4):
            sc = 4 * qc + scl
            ps = opool.tile([128, QD], F32, tag="ot", name=f"pso_{qc}_{scl}")
            for kc in range(DCH):
                nc.tensor.matmul(
                    ps,
                    lhsT=yf[kc][:, 128 * scl : 128 * (scl + 1)],
                    rhs=wout_sb[kc],
                    start=(kc == 0),
                    stop=(kc == DCH - 1),
                )
            osb = osb_pool.tile([128, QD], F32, tag="osb", name=f"osb{qc}_{scl}")
            nc.vector.tensor_copy(out=osb, in_=ps)
            nc.sync.dma_start(out=out_ext[128 * sc : 128 * (sc + 1), :], in_=osb)

    def out_proj_final():
        # final-phase (qc=0) out-proj fed by the two partial-AG buffers:
        # ag_out_fa rows = [g0 hp012 | g1 hp012] -> global y-dim chunks 0,1,2,4,5,6
        # ag_out_fb rows = [g0 hp3   | g1 hp3  ] -> chunks 3,7
        qc = 0
        stages = [
            (ag_out_fa, {0: 0, 1: 128, 2: 256, 4: 384, 5: 512, 6: 640}),
            (ag_out_fb, {3: 0, 7: 128}),
        ]
        ps = [
            opool.tile([128, QD], F32, tag="ot", name=f"psoF_{scl}")
            for scl in range(4)
        ]
        started = set()
        done = set()
        for si, (buf, src) in enumerate(stages):
            yf = {}
            for kc, off in src.items():
                yf[kc] = yf_pool.tile(
                    [128, QCH], BF16, tag=f"yf{kc}", name=f"yfF_{kc}"
                )
                nc.sync.dma_start(out=yf[kc], in_=buf[off : off + 128, :])
            last = si == len(stages) - 1
            for scl in range(4):
                for n, kc in enumerate(sorted(src)):
                    nc.tensor.matmul(
                        ps[scl],
                        lhsT=yf[kc][:, 128 * scl : 128 * (scl + 1)],
                        rhs=wout_sb[kc],
                        start=(scl not in started),
                        stop=(last and n == len(src) - 1),
                    )
                    started.add(scl)
                if last:
                    sc = 4 * qc + scl
                    osb = osb_pool.tile([128, QD], F32, tag="osb", name=f"osbF_{scl}")
                    nc.vector.tensor_copy(out=osb, in_=ps[scl])
                    nc.sync.dma_start(
                        out=out_ext[128 * sc : 128 * (sc + 1), :], in_=osb
                    )

    def ag_final_a():
        nc.gpsimd.collective_compute(
            "AllGather", mybir.AluOpType.bypass, replica_groups=RG,
            ins=[ag_in[0][0:384, :]], outs=[ag_out_fa[:, :]],
        )

    def ag_final_b():
        nc.gpsimd.collective_compute(
            "AllGather", mybir.AluOpType.bypass, replica_groups=RG,
            ins=[ag_in[0][384:512, :]], outs=[ag_out_fb[:, :]],
        )

    # ---------------- schedule ----------------
    nc.gpsimd.collective_compute(
        "AllGather", mybir.AluOpType.bypass, replica_groups=RG,
        ins=[ag_warm_in[:, :]], outs=[ag_warm_out[:, :]],
    )
    prewarm_exp()
    v_proj()
    qk_proj_pair(0)
    qk_proj_pair(1)
    attn_phase(3, inject={0: lambda: qk_proj_pair(2), 1: lambda: qk_proj_pair(3)})
    allgather(3)
    attn_phase(1, inject={2: lambda: out_proj(3)})
    allgather(1)
    attn_phase(2, inject={1: lambda: out_proj(1)})
    allgather(2)
    attn_phase(
        0,
        inject={1: lambda: out_proj(2), 2: ag_final_a, 3: ag_final_b},
    )
    out_proj_final()


_GRAPH = None


def build_graph():
    global _GRAPH
    if _GRAPH is None:
        from contextlib import ExitStack

        nc = bacc.Bacc("TRN2", target_bir_lowering=False)
        with tile.TileContext(nc) as tc, ExitStack() as ctx:
            _build(nc, tc, ctx)
        nc.compile()
        _GRAPH = nc
    return _GRAPH


# ---------------- host-side sharding ----------------

def _perm_rows_for_group(g):
    """wqkv row indices, permuted, for head-group g: Q section then K."""
    rows = []
    for base in (0, D):  # q block, k block
        for j in range(4):  # head pairs
            for hh in (2 * j, 2 * j + 1):
                habs = 8 * g + hh
                rows += list(base + habs * DH + np.arange(0, DH, 2))  # evens
                rows += list(base + habs * DH + np.arange(1, DH, 2))  # odds
    return np.array(rows)


def make_in_maps(x, wqkv, wout):
    x = np.asarray(x, dtype=np.float32)
    wqkv = np.asarray(wqkv, dtype=np.float32)
    wout = np.asarray(wout, dtype=np.float32)

    inv_freq = ROPE_THETA ** (
        -np.arange(0, DH, 2, dtype=np.float32) / np.float32(DH)
    )
    ang = np.arange(S, dtype=np.float32)[:, None] * inv_freq[None, :].astype(
        np.float32
    )
    cosT = np.ascontiguousarray(np.tile(np.cos(ang).T, (4, 1)).astype(bf16))  # [128, S]
    # sign-folded sin: blocks [+sin, -sin, +sin, -sin] so rot = raw*cos + pS
    sin1 = np.sin(ang).T
    sinT = np.ascontiguousarray(
        np.concatenate([sin1, -sin1, sin1, -sin1], axis=0).astype(bf16)
    )

    in_maps = []
    for core in range(NCORES):
        b, g = core // 2, core % 2
        xTb = np.ascontiguousarray(x[b].T.astype(bf16))  # [D, S]
        rows = _perm_rows_for_group(g)
        wqkT = np.ascontiguousarray(wqkv[rows, :].T.astype(bf16))  # [D, 1024]
        vrows = slice(2 * D + 8 * g * DH, 2 * D + (8 * g + 8) * DH)
        wvT = np.ascontiguousarray(wqkv[vrows, :].T.astype(bf16))  # [D, 512]
        woutT = np.ascontiguousarray(
            wout[QD * g : QD * (g + 1), :].T.astype(bf16)
        )  # [D, 512]
        in_maps.append(
            {
                "xT": xTb,
                "wqkT": wqkT,
                "wvT": wvT,
                "woutT": woutT,
                "cosT": cosT,
                "sinT": sinT,
            }
        )
    return in_maps


def assemble(results):
    out = np.empty((B, S, D), dtype=np.float32)
    for core in range(NCORES):
        b, g = core // 2, core % 2
        out[b, :, QD * g : QD * (g + 1)] = results[core]["out"]
    return out


def kernel(x, wqkv, wout, trace=False):
    nc = build_graph()
    in_maps = make_in_maps(x, wqkv, wout)
    res = run_bass_kernel_spmd(nc, in_maps, core_ids=list(range(NCORES)), trace=trace)
    out = assemble(res.results)
    kernel.last_exec_time_ns = res.exec_time_ns
    return out


# revision 23
# speedup vs baseline: 1.0579x; 1.0579x over previous
"""Distributed causal MHA + RoPE kernel for one TRN2 chip (8 NeuronCores).

Problem: x[4, 2048, 1024] f32, wqkv[3072, 1024], wout[1024, 1024],
16 heads, d_head 64, causal, RoPE theta 1e4.

Sharding: core = 2*b + g  (b = batch 0..3, g = head-group 0..1).
Each core computes heads 8g..8g+7 of batch b for the full sequence:
  - QK^T projection in transposed layout (head-dims on partitions,
    positions on free axis), per-head dims permuted [evens|odds] so RoPE
    is plain elementwise ops on contiguous partition ranges.
  - V projection in natural layout [s, dims] with an appended ones
    column, so the PV matmul also produces softmax denominators.
  - S^T = K^T.T @ Q^T per [128k x 512q] chunk, causal chunks skipped,
    exp on ScalarE with the 1/8 scale folded in (no max subtraction --
    scores are provably < ~3 for this data distribution), diagonal
    triangle zeroed with one affine_select per diagonal chunk.
  - O^T accumulated in PSUM, normalized by broadcasted reciprocal
    denominators, written as bf16 y^T.
  - Pairwise (within-batch) AllGather of y^T, overlapped with the next
    phase's attention; out[:, 512g:512g+512] = y_full @ wout.T[:, slice].

v2 schedule: V-projection first (smallest DMA gate), phase order
3,0,2,1 (each AllGather hides under the next phase), one fused
software-pipelined chunk stream per phase with qk-projections /
out-projections injected at head-pair boundaries, and the final phase's
AllGather split in two halves so the tail only waits on the last
128-row exchange.
Output per core: [2048, 512] f32; host concatenates.
"""

import numpy as np
import ml_dtypes

try:
    import concourse.bass as bass
except ImportError:  # staged repo path inside the container
    import sys

    sys.path.insert(0, "/opt/trn_rl_repo")
    import concourse.bass as bass

import concourse.bacc as bacc
import concourse.mybir as mybir
import concourse.tile as tile
from concourse.bass_utils import run_bass_kernel_spmd

bf16 = ml_dtypes.bfloat16

B, S, D, H, DH = 4, 2048, 1024, 16, 64
HPC = H // 2          # heads per core = 8
QD = HPC * DH         # per-core q (or k, or v) dims = 512
NCORES = 8
ROPE_THETA = 10000.0
QCH = 512             # q chunk (matmul moving free dim)
KCH = 128             # k chunk (psum partition dim)
NQC = S // QCH        # 4
NKC = S // KCH        # 16
DCH = D // 128        # 8 contraction chunks for the projections

F32 = mybir.dt.float32
BF16 = mybir.dt.bfloat16

RG = [[0, 1], [2, 3], [4, 5], [6, 7]]


def _build(nc, tc, ctx):
    # ---------------- I/O ----------------
    xT = nc.declare_dram_parameter("xT", [D, S], BF16, isOutput=False)
    wqkT = nc.declare_dram_parameter("wqkT", [D, 2 * QD], BF16, isOutput=False)
    wvT = nc.declare_dram_parameter("wvT", [D, QD], BF16, isOutput=False)
    woutT = nc.declare_dram_parameter("woutT", [D, QD], BF16, isOutput=False)
    cosT = nc.declare_dram_parameter("cosT", [128, S], BF16, isOutput=False)
    sinT = nc.declare_dram_parameter("sinT", [128, S], BF16, isOutput=False)
    out_ext = nc.declare_dram_parameter("out", [S, QD], F32, isOutput=True)

    # AllGather bounce buffers. Full-phase AGs for qc 3,0,2; the final
    # phase (qc=1) exchanges in two halves (hp01 then hp23).
    ag_in = []
    ag_out = []
    for ph in range(NQC):
        ag_in.append(
            tc.tile([QD, QCH], BF16, space="DRAM", name=f"ag_in{ph}", bufs=1)[0]
        )
        ag_out.append(
            tc.tile(
                [2 * QD, QCH],
                BF16,
                space="DRAM",
                addr_space="Shared",
                name=f"ag_out{ph}",
                bufs=1,
            )[0]
        )
    ag_out_fa = tc.tile(
        [6 * KCH, QCH], BF16, space="DRAM", addr_space="Shared", name="ag_out_fa", bufs=1
    )[0]
    ag_out_fb = tc.tile(
        [2 * KCH, QCH], BF16, space="DRAM", addr_space="Shared", name="ag_out_fb", bufs=1
    )[0]
    # Tiny warm-up collective: the first collective of the NEFF absorbs the
    # cross-core launch skew + ncfw setup (~40-60us observed). Fire it at
    # t~0 with no data dependencies so the cost overlaps the projections.
    ag_warm_in = tc.tile([1, 16], BF16, space="DRAM", name="ag_warm_in", bufs=1)[0]
    ag_warm_out = tc.tile(
        [2, 16], BF16, space="DRAM", addr_space="Shared", name="ag_warm_out", bufs=1
    )[0]

    # ---------------- SBUF pools ----------------
    const_pool = ctx.enter_context(tc.tile_pool(name="consts", bufs=1))
    xt_sb = [const_pool.tile([128, S], BF16, name=f"xt{i}") for i in range(DCH)]
    wqk_sb = [const_pool.tile([128, 2 * QD], BF16, name=f"wqk{i}") for i in range(DCH)]
    wv_sb = [const_pool.tile([128, QD], BF16, name=f"wv{i}") for i in range(DCH)]
    wout_sb = [const_pool.tile([128, QD], BF16, name=f"wo{i}") for i in range(DCH)]
    cos_sb = const_pool.tile([128, S], BF16, name="cos")
    sin_sb = const_pool.tile([128, S], BF16, name="sin")

    # --- initial DMA, priming order: v-proj inputs first, wout last.
    # Issues are spread across the sync and scalar (both hardware-DGE)
    # descriptor queues -- each dma_start costs ~0.7us of issue time on
    # its queue, so a single queue serializes the whole load.
    for i in range(DCH):
        r = slice(128 * i, 128 * (i + 1))
        nc.sync.dma_start(out=wv_sb[i], in_=wvT[r, :])
        nc.sync.dma_start(out=xt_sb[i][:, 0:QCH], in_=xT[r, 0:QCH])
    for c in (1, 2):  # scalar queue: xt columns for v-proj sc 4..11
        for i in range(DCH):
            nc.scalar.dma_start(
                out=xt_sb[i][:, QCH * c : QCH * (c + 1)],
                in_=xT[128 * i : 128 * (i + 1), QCH * c : QCH * (c + 1)],
            )
    for i in range(DCH):  # sync queue: xt c3
        nc.sync.dma_start(
            out=xt_sb[i][:, 3 * QCH : 4 * QCH],
            in_=xT[128 * i : 128 * (i + 1), 3 * QCH : 4 * QCH],
        )
    for c in (0, 2, 1, 3):  # scalar queue: wqk (pairs 0,1 first)
        for i in range(DCH):
            nc.scalar.dma_start(
                out=wqk_sb[i][:, 256 * c : 256 * (c + 1)],
                in_=wqkT[128 * i : 128 * (i + 1), 256 * c : 256 * (c + 1)],
            )
    for c in range(4):
        nc.sync.dma_start(
            out=cos_sb[:, QCH * c : QCH * (c + 1)],
            in_=cosT[:, QCH * c : QCH * (c + 1)],
        )
        nc.sync.dma_start(
            out=sin_sb[:, QCH * c : QCH * (c + 1)],
            in_=sinT[:, QCH * c : QCH * (c + 1)],
        )
    for i in range(DCH):
        nc.sync.dma_start(out=wout_sb[i], in_=woutT[128 * i : 128 * (i + 1), :])

    # lower-triangular (keep q >= k) [128, 128] mask, built once
    tri_sb = const_pool.tile([KCH, KCH], BF16, name="tri")
    nc.vector.memset(tri_sb, 1.0)
    nc.gpsimd.affine_select(
        out=tri_sb,
        in_=tri_sb,
        pattern=[[1, KCH]],
        channel_multiplier=-1,
        base=0,
        compare_op=mybir.AluOpType.is_ge,
        fill=0.0,
    )

    # persistent activation buffers
    qkrot_pool = ctx.enter_context(tc.tile_pool(name="qkrot", bufs=1))
    qkrot = [qkrot_pool.tile([128, S], BF16, name=f"qkrot{m}") for m in range(8)]
    vaug_pool = ctx.enter_context(tc.tile_pool(name="vaug", bufs=1))
    v_sb = [vaug_pool.tile([128, HPC, DH + 1], BF16, name=f"v{sc}") for sc in range(16)]
    ybuf_pool = ctx.enter_context(tc.tile_pool(name="ybuf", bufs=1))
    ybuf = [ybuf_pool.tile([128, S], BF16, name=f"y{hp}") for hp in range(4)]

    # transient pools
    spool = ctx.enter_context(tc.tile_pool(name="spsum", bufs=2, space="PSUM"))
    opool = ctx.enter_context(tc.tile_pool(name="opsum", bufs=4, space="PSUM"))
    qkraw_pool = ctx.enter_context(tc.tile_pool(name="qkraw", bufs=2))
    rope_tmp = ctx.enter_context(tc.tile_pool(name="ropetmp", bufs=2))
    p_pool = ctx.enter_context(tc.tile_pool(name="ptiles", bufs=6))
    r_pool = ctx.enter_context(tc.tile_pool(name="rbcast", bufs=2))
    yf_pool = ctx.enter_context(tc.tile_pool(name="yfull", bufs=2))
    osb_pool = ctx.enter_context(tc.tile_pool(name="outsb", bufs=3))
    warm_pool = ctx.enter_context(tc.tile_pool(name="warm", bufs=1))

    # preload the scalar-engine exp table before any real work
    def prewarm_exp():
        wsrc = warm_pool.tile([1, 8], F32, name="wsrc")
        wdst = warm_pool.tile([1, 8], BF16, name="wdst")
        nc.vector.memset(wsrc, 0.0)
        nc.scalar.activation(
            out=wdst, in_=wsrc,
            func=mybir.ActivationFunctionType.Exp, scale=0.125,
        )

    # ---------------- V projection (natural layout + ones column) -------
    def v_proj():
        for sc in range(16):
            ps = opool.tile([128, QD], F32, tag="ot", name=f"psv_{sc}")
            for kc in range(DCH):
                nc.tensor.matmul(
                    ps,
                    lhsT=xt_sb[kc][:, 128 * sc : 128 * (sc + 1)],
                    rhs=wv_sb[kc],
                    start=(kc == 0),
                    stop=(kc == DCH - 1),
                )
            nc.vector.tensor_copy(
                out=v_sb[sc][:, :, 0:DH],
                in_=ps.rearrange("p (h d) -> p h d", h=HPC),
            )
            nc.vector.memset(v_sb[sc][:, :, DH : DH + 1], 1.0)

    # ---------------- QK^T projection + RoPE for head-pair j ----------------
    def qk_proj_pair(j):
        raws = {}
        for m in (j, 4 + j):
            raws[m] = qkraw_pool.tile([128, S], BF16, tag="raw", name=f"raw{m}")
        for nch in range(NQC):
            for m in (j, 4 + j):
                ps = opool.tile([128, QCH], F32, tag="ot", name=f"psqk_{m}_{nch}")
                for kc in range(DCH):
                    nc.tensor.matmul(
                        ps,
                        lhsT=wqk_sb[kc][:, 128 * m : 128 * (m + 1)],
                        rhs=xt_sb[kc][:, QCH * nch : QCH * (nch + 1)],
                        start=(kc == 0),
                        stop=(kc == DCH - 1),
                    )
                nc.scalar.copy(
                    out=raws[m][:, QCH * nch : QCH * (nch + 1)], in_=ps
                )
        for m in (j, 4 + j):
            raw = raws[m]
            # RoPE, 6 DVE ops: rot = raw*cosF + pS (sin signs folded in sin_sb)
            pC = rope_tmp.tile([128, S], BF16, tag="pC", name=f"pC_{m}")
            pS = rope_tmp.tile([128, S], BF16, tag="pS", name=f"pS_{m}")
            nc.vector.tensor_mul(pC, raw, cos_sb)
            nc.vector.tensor_mul(pS[0:32, :], raw[32:64, :], sin_sb[32:64, :])
            nc.vector.tensor_mul(pS[32:64, :], raw[0:32, :], sin_sb[0:32, :])
            nc.vector.tensor_mul(pS[64:96, :], raw[96:128, :], sin_sb[96:128, :])
            nc.vector.tensor_mul(pS[96:128, :], raw[64:96, :], sin_sb[64:96, :])
            nc.vector.tensor_add(qkrot[m], pC, pS)

    # ---------------- attention phase: fused pipelined chunk stream ------
    # One phase = all 4 head-pairs of q-chunk qc, software-pipelined with
    # lag 1 ACROSS hp boundaries:  S(i) exp(i) PV(i-1) ... so the scalar
    # engine always has exp work one chunk ahead of the PE's PV.
    # `inject[hp]` closures (projection / out-projection blocks) are
    # emitted right after hp's normalize, at which point that hp's PSUM
    # accumulators have been freed.
    def attn_phase(qc, inject=None):
        inject = inject or {}
        nkc = 4 * qc + 4
        chunks = [(hp, kc) for hp in range(4) for kc in range(nkc)]
        o_ps = {}
        s_tiles = {}
        p_tiles = {}

        def emit_S(hp, kc):
            c0 = max(0, KCH * kc - QCH * qc)
            s_ps = spool.tile(
                [128, 2 * QCH], F32, tag="stile", name=f"s_{qc}_{hp}_{kc}"
            )
            for ab in range(2):
                lo = 64 * ab
                nc.tensor.matmul(
                    s_ps[:, QCH * ab + c0 : QCH * (ab + 1)],
                    lhsT=qkrot[4 + hp][lo : lo + 64, KCH * kc : KCH * (kc + 1)],
                    rhs=qkrot[hp][lo : lo + 64, QCH * qc + c0 : QCH * (qc + 1)],
                    start=True,
                    stop=True,
                )
            s_tiles[(hp, kc)] = s_ps

        def emit_exp(hp, kc):
            c0 = max(0, KCH * kc - QCH * qc)
            s_ps = s_tiles.pop((hp, kc))
            p_t = p_pool.tile(
                [128, 2 * QCH], BF16, tag="ptile", name=f"p_{qc}_{hp}_{kc}"
            )
            if c0 == 0:
                nc.scalar.activation(
                    out=p_t, in_=s_ps,
                    func=mybir.ActivationFunctionType.Exp, scale=0.125,
                )
            else:
                for ab in range(2):
                    nc.scalar.activation(
                        out=p_t[:, QCH * ab + c0 : QCH * (ab + 1)],
                        in_=s_ps[:, QCH * ab + c0 : QCH * (ab + 1)],
                        func=mybir.ActivationFunctionType.Exp, scale=0.125,
                    )
            if KCH * kc >= QCH * qc:  # diagonal: zero k > q
                for ab in range(2):
                    nc.vector.tensor_mul(
                        p_t[:, QCH * ab + c0 : QCH * ab + c0 + KCH],
                        p_t[:, QCH * ab + c0 : QCH * ab + c0 + KCH],
                        tri_sb,
                    )
            p_tiles[(hp, kc)] = p_t

        def emit_PV(hp, kc):
            c0 = max(0, KCH * kc - QCH * qc)
            if kc == 0:
                o_ps[hp] = [
                    opool.tile([DH + 1, QCH], F32, tag="ot", name=f"o_{qc}_{hp}_{ab}")
                    for ab in range(2)
                ]
            p_t = p_tiles.pop((hp, kc))
            for ab in range(2):
                nc.tensor.matmul(
                    o_ps[hp][ab][:, c0:QCH],
                    lhsT=v_sb[kc][:, 2 * hp + ab, :],
                    rhs=p_t[:, QCH * ab + c0 : QCH * (ab + 1)],
                    start=(kc == 0),
                    stop=(kc == nkc - 1),
                )

        def emit_norm(hp):
            ops = o_ps.pop(hp)
            for ab in range(2):
                den0 = r_pool.tile([1, QCH], F32, tag="den0", name=f"dn{qc}_{hp}_{ab}")
                nc.vector.tensor_copy(out=den0, in_=ops[ab][DH : DH + 1, :])
                rsrc = r_pool.tile([1, QCH], F32, tag="rsrc", name=f"rs{qc}_{hp}_{ab}")
                nc.vector.reciprocal_approx_fast(out=rsrc, in_=den0)
                rbc = r_pool.tile([DH, QCH], F32, tag="rbc", name=f"rb{qc}_{hp}_{ab}")
                nc.gpsimd.partition_broadcast(rbc, rsrc)
                nc.vector.tensor_mul(
                    ybuf[hp][64 * ab : 64 * ab + 64, QCH * qc : QCH * (qc + 1)],
                    ops[ab][0:DH, :],
                    rbc,
                )
            nc.sync.dma_start(
                out=ag_in[qc][128 * hp : 128 * (hp + 1), :],
                in_=ybuf[hp][:, QCH * qc : QCH * (qc + 1)],
            )

        prev = None
        for hp, kc in chunks:
            emit_S(hp, kc)
            emit_exp(hp, kc)
            if prev is not None:
                emit_PV(*prev)
                if prev[1] == nkc - 1:
                    emit_norm(prev[0])
                    if prev[0] in inject:
                        inject[prev[0]]()
            prev = (hp, kc)
        emit_PV(*prev)
        emit_norm(prev[0])
        if prev[0] in inject:
            inject[prev[0]]()

    def allgather(qc):
        nc.gpsimd.collective_compute(
            "AllGather",
            mybir.AluOpType.bypass,
            replica_groups=RG,
            ins=[ag_in[qc][:, :]],
            outs=[ag_out[qc][:, :]],
        )

    def out_proj(qc):
        yf = [
            yf_pool.tile([128, QCH], BF16, tag=f"yf{i}", name=f"yf{qc}_{i}")
            for i in range(DCH)
        ]
        for i in range(DCH):
            nc.sync.dma_start(out=yf[i], in_=ag_out[qc][128 * i : 128 * (i + 1), :])
        for scl in range(4):
            sc = 4 * qc + scl
            ps = opool.tile([128, QD], F32, tag="ot", name=f"pso_{qc}_{scl}")
            for kc in range(DCH):
                nc.tensor.matmul(
                    ps,
                    lhsT=yf[kc][:, 128 * scl : 128 * (scl + 1)],
                    rhs=wout_sb[kc],
                    start=(kc == 0),
                    stop=(kc == DCH - 1),
                )
            osb = osb_pool.tile([128, QD], F32, tag="osb", name=f"osb{qc}_{scl}")
            nc.vector.tensor_copy(out=osb, in_=ps)
            nc.sync.dma_start(out=out_ext[128 * sc : 128 * (sc + 1), :], in_=osb)

    def out_proj_final():
        # final-phase (qc=1) out-proj fed by the two partial-AG buffers:
        # ag_out_fa rows = [g0 hp012 | g1 hp012] -> global y-dim chunks 0,1,2,4,5,6
        # ag_out_fb rows = [g0 hp3   | g1 hp3  ] -> chunks 3,7
        qc = 1
        stages = [
            (ag_out_fa, {0: 0, 1: 128, 2: 256, 4: 384, 5: 512, 6: 640}),
            (ag_out_fb, {3: 0, 7: 128}),
        ]
        ps = [
            opool.tile([128, QD], F32, tag="ot", name=f"psoF_{scl}")
            for scl in range(4)
        ]
        started = set()
        done = set()
        for si, (buf, src) in enumerate(stages):
            yf = {}
            for kc, off in src.items():
                yf[kc] = yf_pool.tile(
                    [128, QCH], BF16, tag=f"yf{kc}", name=f"yfF_{kc}"
                )
                nc.sync.dma_start(out=yf[kc], in_=buf[off : off + 128, :])
            last = si == len(stages) - 1
            for scl in range(4):
                for n, kc in enumerate(sorted(src)):
                    nc.tensor.matmul(
                        ps[scl],
                        lhsT=yf[kc][:, 128 * scl : 128 * (scl + 1)],
                        rhs=wout_sb[kc],
                        start=(scl not in started),
                        stop=(last and n == len(src) - 1),
                    )
                    started.add(scl)
                if last:
                    sc = 4 * qc + scl
                    osb = osb_pool.tile([128, QD], F32, tag="osb", name=f"osbF_{scl}")
                    nc.vector.tensor_copy(out=osb, in_=ps[scl])
                    nc.sync.dma_start(
                        out=out_ext[128 * sc : 128 * (sc + 1), :], in_=osb
                    )

    def ag_final_a():
        nc.gpsimd.collective_compute(
            "AllGather", mybir.AluOpType.bypass, replica_groups=RG,
            ins=[ag_in[1][0:384, :]], outs=[ag_out_fa[:, :]],
        )

    def ag_final_b():
        nc.gpsimd.collective_compute(
            "AllGather", mybir.AluOpType.bypass, replica_groups=RG,
            ins=[ag_in[1][384:512, :]], outs=[ag_out_fb[:, :]],
        )

    # ---------------- schedule ----------------
    nc.gpsimd.collective_compute(
        "AllGather", mybir.AluOpType.bypass, replica_groups=RG,
        ins=[ag_warm_in[:, :]], outs=[ag_warm_out[:, :]],
    )
    prewarm_exp()
    v_proj()
    qk_proj_pair(0)
    qk_proj_pair(1)
    attn_phase(3, inject={0: lambda: qk_proj_pair(2), 1: lambda: qk_proj_pair(3)})
    allgather(3)
    attn_phase(0, inject={3: lambda: out_proj(3)})
    allgather(0)
    attn_phase(2, inject={2: lambda: out_proj(0)})
    allgather(2)
    attn_phase(
        1,
        inject={2: lambda: (ag_final_a(), out_proj(2)), 3: ag_final_b},
    )
    out_proj_final()


_GRAPH = None


def build_graph():
    global _GRAPH
    if _GRAPH is None:
        from contextlib import ExitStack

        nc = bacc.Bacc("TRN2", target_bir_lowering=False)
        with tile.TileContext(nc) as tc, ExitStack() as ctx:
            _build(nc, tc, ctx)
        nc.compile()
        _GRAPH = nc
    return _GRAPH


# ---------------- host-side sharding ----------------

def _perm_rows_for_group(g):
    """wqkv row indices, permuted, for head-group g: Q section then K."""
    rows = []
    for base in (0, D):  # q block, k block
        for j in range(4):  # head pairs
            for hh in (2 * j, 2 * j + 1):
                habs = 8 * g + hh
                rows += list(base + habs * DH + np.arange(0, DH, 2))  # evens
                rows += list(base + habs * DH + np.arange(1, DH, 2))  # odds
    return np.array(rows)


def make_in_maps(x, wqkv, wout):
    x = np.asarray(x, dtype=np.float32)
    wqkv = np.asarray(wqkv, dtype=np.float32)
    wout = np.asarray(wout, dtype=np.float32)

    inv_freq = ROPE_THETA ** (
        -np.arange(0, DH, 2, dtype=np.float32) / np.float32(DH)
    )
    ang = np.arange(S, dtype=np.float32)[:, None] * inv_freq[None, :].astype(
        np.float32
    )
    cosT = np.ascontiguousarray(np.tile(np.cos(ang).T, (4, 1)).astype(bf16))  # [128, S]
    # sign-folded sin: blocks [+sin, -sin, +sin, -sin] so rot = raw*cos + pS
    sin1 = np.sin(ang).T
    sinT = np.ascontiguousarray(
        np.concatenate([sin1, -sin1, sin1, -sin1], axis=0).astype(bf16)
    )

    in_maps = []
    for core in range(NCORES):
        b, g = core // 2, core % 2
        xTb = np.ascontiguousarray(x[b].T.astype(bf16))  # [D, S]
        rows = _perm_rows_for_group(g)
        wqkT = np.ascontiguousarray(wqkv[rows, :].T.astype(bf16))  # [D, 1024]
        vrows = slice(2 * D + 8 * g * DH, 2 * D + (8 * g + 8) * DH)
        wvT = np.ascontiguousarray(wqkv[vrows, :].T.astype(bf16))  # [D, 512]
        woutT = np.ascontiguousarray(
            wout[QD * g : QD * (g + 1), :].T.astype(bf16)
        )  # [D, 512]
        in_maps.append(
            {
                "xT": xTb,
                "wqkT": wqkT,
                "wvT": wvT,
                "woutT": woutT,
                "cosT": cosT,
                "sinT": sinT,
            }
        )
    return in_maps


def assemble(results):
    out = np.empty((B, S, D), dtype=np.float32)
    for core in range(NCORES):
        b, g = core // 2, core % 2
        out[b, :, QD * g : QD * (g + 1)] = results[core]["out"]
    return out


def kernel(x, wqkv, wout, trace=False):
    nc = build_graph()
    in_maps = make_in_maps(x, wqkv, wout)
    res = run_bass_kernel_spmd(nc, in_maps, core_ids=list(range(NCORES)), trace=trace)
    out = assemble(res.results)
    kernel.last_exec_time_ns = res.exec_time_ns
    return out
